# revision 108
# baseline (speedup 1.0000x reference)
"""Causal self-attention (b=4, s=2048, d=1024, 16 heads) on 8 trn2 NeuronCores.

Sharding: core j <- heads {2j, 2j+1} for ALL batches (tensor-parallel over
heads).  Each core projects q/k/v for its 2 heads over all 4 batches, runs
causal attention for them, then the 8 cores exchange attention outputs with
one 8-way AllToAll per 512-token chunk: core j receives the full 1024-channel
attention output for its output-token slice (batch j//2, query-tile parity
j%2) and computes the full output projection for that slice (no duplicated
FLOPs anywhere, and AllToAll moves half the bytes an AllGather would).

Schedule: attention is exp(ACT)-throughput-paced, so the q/k/v projection
chains for chunk r+1 (and, in chunk 3, the earlier rounds' output
projections) are emitted as per-iteration fillers inside chunk r's
attention loops to keep the PE busy during the per-tile exp bubbles.  The
attn@v matmuls run FOUR tiles behind their scores, with the pend queue
carried ACROSS block boundaries so the exp pipeline never drains; each
block's a2a staging is therefore deferred into the next block's tile-3
slot (its last psum drains are only emitted there).

Layouts:
  - x is fed pre-transposed: x_t [1024, 4, 2048] (c-major per batch); each
    (batch, chunk) x block loads as ONE strided dma into [128, 8ct, 512].
  - q^T, k^T come out of the projection as [feat, token]; v comes out
    token-major [token, head, 65] with a ones column per head so the attn@v
    matmul also accumulates the softmax denominator as an extra feature.
  - scores^T tiles are [tk, tq]; softmax runs without max-subtraction
    (scores bounded for this distribution); the two heads run concurrently
    in PE row groups 0-63 / 64-127 sharing one psum tile / one exp; the
    causal mask is a multiply on the diagonal 128-column block only.
  - attn@v runs TRANSPOSED: lhsT = e-tile [keys, 128-query block], rhs =
    v [keys, 65] -> psum [128 queries, 65] per (head, query-block).  The PE
    cost charges free-dim size only, so this streams 65 columns instead of
    512 per tile: ~2x fewer PE cycles for attention@v.  The denominator
    lands per-query-partition (col 64), so normalization is a per-partition
    tensor_scalar (no partition_broadcast).
  - the AllToAll payload is TOKEN-major (block j = [256 tokens of slice j,
    my 128 feats]) staged directly from the normalized attn@v output -- no
    transposes on the attention path.  The gathered blocks are flipped
    feature-major by 128-col PE transposes overlapped with the outproj.

PSUM accumulation-group discipline (hardware-validated): a matmul with
start=True bumps its psum BANK's epoch; any OTHER open group in that bank
then overwrites (not accumulates) on its next write.  So every bank gets
exactly ONE start (its first matmul of the block) and one stop (its last),
with all interleaved matmuls accumulating inside that single group.

All matmuls run bf16 operands with fp32 psum accumulation.
"""

import numpy as np

N_HEADS = 16
B = 4
S = 2048
C = 1024
HD = C // N_HEADS            # 64
N_CORES = 8
H_LOC = 2                    # heads per core
F_LOC = H_LOC * HD           # 128 local qkv features
P = 128                      # partitions
NCT = C // P                 # 8 contraction tiles over channels
NTT = S // P                 # 16 token tiles
TQ = 512                     # query-chunk width (one psum bank)
NQ = S // TQ                 # 4 query chunks
QTPC = TQ // P               # 4 query tiles per chunk
SL_T = S // 2                # 1024 tokens per output slice
SCALE = 1.0 / float(np.sqrt(HD))

_NC_CACHE = {}


def _build_nc(debug=False):
    import concourse.bacc as bacc
    import concourse.tile as tile
    from concourse import mybir
    from concourse.bass import _add_dep_helper

    dt = mybir.dt
    f32, bf16 = dt.float32, dt.bfloat16
    EXP = mybir.ActivationFunctionType.Exp
    GE = mybir.AluOpType.is_ge
    BYP = mybir.AluOpType.bypass
    GROUP8 = [list(range(N_CORES))]

    nc = bacc.Bacc("TRN2", num_devices=N_CORES)

    x_t = nc.dram_tensor("x_t", [C, B, S], bf16, kind="ExternalInput")
    w_q = nc.dram_tensor("w_q", [C, F_LOC], bf16, kind="ExternalInput")
    w_k = nc.dram_tensor("w_k", [C, F_LOC], bf16, kind="ExternalInput")
    w_v = nc.dram_tensor("w_v", [C, F_LOC], bf16, kind="ExternalInput")
    w_p = nc.dram_tensor("w_p", [C, C], bf16, kind="ExternalInput")
    # my slice: 8 qtiles (parity-interleaved), 2 per chunk, full channels
    out = nc.dram_tensor("out", [SL_T, C], f32, kind="ExternalOutput")
    nao_dump = rec_dump = None
    if debug:
        nao_dump = nc.dram_tensor("nao_dump", [NQ, B, P, 2, 2, 2, HD],
                                  bf16, kind="ExternalOutput")
        rec_dump = nc.dram_tensor("rec_dump", [NQ, B, P, QTPC, 2, 1],
                                  f32, kind="ExternalOutput")

    x_r = x_t.rearrange("(c p) b s -> p c b s", p=P)

    with tile.TileContext(nc) as tc:
        with (
            tc.tile_pool(name="persist", bufs=1) as persist,
            tc.tile_pool(name="xqpool", bufs=2) as xqpool,
            tc.tile_pool(name="qpool", bufs=2) as qpool,
            tc.tile_pool(name="epool", bufs=6) as epool,
            tc.tile_pool(name="npool", bufs=2) as npool,
            tc.tile_pool(name="agpool", bufs=3) as agpool,
            tc.tile_pool(name="fpool", bufs=2) as fpool,
            tc.tile_pool(name="psmm", bufs=2, space="PSUM") as psmm,
            tc.tile_pool(name="psav", bufs=1, space="PSUM") as psav,
            tc.tile_pool(name="pspr", bufs=2, space="PSUM") as pspr,
            tc.tile_pool(name="drpool", bufs=1, space="DRAM") as drpool,
        ):
            # ---- weights: one dma each, [C, F] dram -> [P, NCT, F] sbuf ----
            def load_w(wdram, nm, F, eng):
                t = persist.tile([P, NCT, F], bf16, name=nm, tag=nm)
                eng.dma_start(
                    out=t, in_=wdram.rearrange("(c p) f -> p c f", p=P))
                return [t[:, ct, :] for ct in range(NCT)]

            def load_xq(b, r, eng):
                t = xqpool.tile([P, NCT, TQ], bf16, name=f"xq{b}_{r}",
                                tag=f"xq{b}")
                eng.dma_start(out=t,
                              in_=x_r[:, :, b, r * TQ:(r + 1) * TQ])
                return t

            # w_q and the first x block lead (parallel queues) so the first
            # chain starts early
            wq_sb = load_w(w_q, "wq", F_LOC, nc.scalar)
            xq00 = xqpool.tile([P, NCT, TQ], bf16, name="xq0_0", tag="xq0")
            nc.sync.dma_start(out=xq00[:, 0:4, :],
                              in_=x_r[:, 0:4, 0, 0:TQ])
            nc.sync.dma_start(out=xq00[:, 4:NCT, :],
                              in_=x_r[:, 4:NCT, 0, 0:TQ])
            xq0 = [xq00]
            wk_sb = load_w(w_k, "wk", F_LOC, nc.gpsimd)
            wv_sb = load_w(w_v, "wv", F_LOC, nc.scalar)
            wp_sb = []
            xq0 += [load_xq(b, 0, (nc.sync, nc.scalar)[b % 2])
                    for b in range(1, B)]

            # ---- PE program order is pinned to emission order ----
            prev_mm = [None]

            def mm(*a, **k):
                m = nc.tensor.matmul(*a, **k)
                if prev_mm[0] is not None:
                    _add_dep_helper(m.ins, prev_mm[0], sync=False,
                                    reason="pe emission order")
                prev_mm[0] = m.ins
                return m

            # near-dep-free junk operand (first DVE instruction) so the
            # warm-up matmuls issue almost immediately
            junk = persist.tile([P, TQ], bf16, name="junk", tag="junk")
            nc.vector.memset(junk[:, 0:1], 0.0)

            # ---- warm-up: junk matmuls ramp the PE p-state while the
            # first loads are in flight (results are never read) ----
            for i in range(12):
                dmy = pspr.tile([P, TQ], f32, name=f"dmw{i}", tag="pp")
                mm(dmy, lhsT=junk[:, 0:P], rhs=junk, start=True, stop=True)

            # additive causal mask for the diagonal 128x128 block, applied
            # in the scores PSUM accumulation: out[k,q] += neg_tri[q,k]
            # via matmul(lhsT=neg_tri, rhs=ident) -- no vector-engine hop
            # on the exp->attn@v critical path.  -240*SCALE = -30 => exp==0.
            neg_tri = persist.tile([P, P], bf16, name="neg_tri",
                                   tag="neg_tri")
            nc.gpsimd.memset(neg_tri, -240.0)
            nc.gpsimd.affine_select(
                out=neg_tri, in_=neg_tri, compare_op=GE, fill=0.0,
                base=-1, pattern=[[1, P]], channel_multiplier=-1)

            # multiply-mask fallback for the diagonal block (debug)
            mask_tri = persist.tile([P, 2, P], bf16, name="mask_tri",
                                    tag="mask_tri")
            nc.gpsimd.memset(mask_tri, 1.0)
            nc.gpsimd.affine_select(
                out=mask_tri, in_=mask_tri, compare_op=GE, fill=0.0,
                base=0, pattern=[[0, 2], [1, P]], channel_multiplier=-1)

            # identity for PE transposes of the gathered a2a blocks
            ident = persist.tile([P, P], bf16, name="ident", tag="ident")
            nc.gpsimd.memset(ident, 1.0)
            nc.gpsimd.affine_select(
                out=ident, in_=ident, compare_op=mybir.AluOpType.is_equal,
                fill=0.0, base=0, pattern=[[1, P]], channel_multiplier=-1)

            kT = [persist.tile([P, S], bf16, name=f"kT{b}", tag=f"kT{b}")
                  for b in range(B)]
            qT = [[None] * NQ for _ in range(B)]
            # v, token-major, ones column per head: [token, head, 65]
            v_sb = [[persist.tile([P, H_LOC, HD + 1], bf16, name=f"v{b}_{tt}",
                                  tag=f"v{b}_{tt}")
                     for tt in range(NTT)] for b in range(B)]
            for b in range(B):
                for tt in range(NTT):
                    eng = nc.vector if (tt % 2 == 0) else nc.gpsimd
                    eng.memset(v_sb[b][tt][:, :, HD:HD + 1], 1.0)

            # AllToAll bounce buffers, TOKEN-major: block j (rows 256j..)
            # = [256 tokens of slice j = (batch j//2, parity j%2), my 128
            # feats].  Staged directly from the normalized attn@v output
            # (token-major), so no transposes on the attention path; the
            # gathered side transposes while overlapped with outproj.
            a2a_in = [drpool.tile([N_CORES * 2 * P, P], bf16,
                                  name=f"a2a_in{r}", tag=f"a2a_in{r}")
                      for r in range(NQ)]
            a2a_out = [drpool.tile([N_CORES * 2 * P, P], bf16,
                                   name=f"a2a_out{r}", tag=f"a2a_out{r}")
                       for r in range(NQ)]

            # ---- projection chain emitters (used as fillers) ----
            def proj_q(b, r, xq):
                # filler generator: yields after each PE matmul
                ps = pspr.tile([P, TQ], f32, name=f"ps_q{b}_{r}", tag="pp")
                for ct in range(NCT):
                    mm(ps, lhsT=wq_sb[ct], rhs=xq[:, ct, :],
                       start=(ct == 0), stop=(ct == NCT - 1))
                    yield
                t = qpool.tile([P, TQ], bf16, name=f"qT{b}_{r}",
                               tag=f"q{b}")
                qT[b][r] = t
                nc.vector.tensor_copy(t, ps)

            def proj_k(b, r, xq):
                ps = pspr.tile([P, TQ], f32, name=f"ps_k{b}_{r}", tag="pp")
                for ct in range(NCT):
                    mm(ps, lhsT=wk_sb[ct], rhs=xq[:, ct, :],
                       start=(ct == 0), stop=(ct == NCT - 1))
                    yield
                nc.vector.tensor_copy(kT[b][:, r * TQ:(r + 1) * TQ], ps)

            def proj_v(b, r, xq):
                ps = pspr.tile([P, TQ], f32, name=f"ps_v{b}_{r}", tag="pp")
                for tt in range(QTPC):
                    for ct in range(NCT):
                        mm(ps[:, tt * F_LOC:(tt + 1) * F_LOC],
                           lhsT=xq[:, ct, tt * P:(tt + 1) * P],
                           rhs=wv_sb[ct],
                           start=(ct == 0), stop=(ct == NCT - 1))
                        yield
                for tt in range(QTPC):
                    nc.vector.tensor_copy(
                        v_sb[b][r * QTPC + tt][:, :, 0:HD],
                        ps[:, tt * F_LOC:(tt + 1) * F_LOC].rearrange(
                            "p (h d) -> p h d", h=H_LOC))

            def proj_v_inline(b, r, xq):
                # same-block-safe: each tt's v-tile copy is emitted before
                # the final yield of its chain, so consumers emitted later
                # in this block are ordered after the write
                ps = pspr.tile([P, TQ], f32, name=f"ps_v{b}_{r}", tag="pp")
                for tt in range(QTPC):
                    for ct in range(NCT):
                        mm(ps[:, tt * F_LOC:(tt + 1) * F_LOC],
                           lhsT=xq[:, ct, tt * P:(tt + 1) * P],
                           rhs=wv_sb[ct],
                           start=(ct == 0), stop=(ct == NCT - 1))
                        if ct < NCT - 1:
                            yield
                    nc.vector.tensor_copy(
                        v_sb[b][r * QTPC + tt][:, :, 0:HD],
                        ps[:, tt * F_LOC:(tt + 1) * F_LOC].rearrange(
                            "p (h d) -> p h d", h=H_LOC))
                    yield

            def proj_all(b, r, xq):
                yield from proj_q(b, r, xq)
                yield from proj_k(b, r, xq)
                yield from proj_v(b, r, xq)

            # ---- prelude: chunk-0 projections for batch 0 only; batches
            # 1-3 weave into the chunk-0 attention blocks as fillers so
            # the PE keeps pace with the serial x-load DMAs ----
            for _ in proj_all(0, 0, xq0[0]):
                pass

            # ---- main loop: attention with proj/outproj fillers + rounds --
            aog_map = {}
            aogT_map = {}
            po_map = {}

            def outproj_reloads(rp, order_gate, split=False):
                # reload gathered a2a_out[rp] token-major: [P tok, feat
                # block f, tok tile t, 128 feats]; two dmas (t=0 first)
                # when latency-critical; gated so the scheduler can't
                # hoist it to where it'd block a queue on the collective
                t = agpool.tile([P, NCT, 2, P], bf16, name=f"aog{rp}",
                                tag="aog")
                a2r = a2a_out[rp].rearrange("(f tt p) c -> p f tt c", p=P,
                                            tt=2)
                if split:
                    d = nc.sync.dma_start(out=t[:, :, 0, :],
                                          in_=a2r[:, :, 0, :])
                    nc.sync.dma_start(out=t[:, :, 1, :], in_=a2r[:, :, 1, :])
                else:
                    d = nc.sync.dma_start(out=t, in_=a2r)
                if order_gate is not None:
                    _add_dep_helper(d.ins, order_gate, sync=False,
                                    reason="reload after collective post")
                aog_map[rp] = t

            def gen_transposes(rp, use_act=False, qts=(0, 1)):
                # PE-transpose the gathered token-major blocks of round rp
                # back to feature-major (16 x 128-col transposes).  The
                # psum->sbuf copies pair two ct blocks and alternate
                # DVE/ACT when use_act (post-loop: ACT has no exp work)
                aog = aog_map[rp]
                for qt in qts:
                    aogT = agpool.tile([P, NCT, P], bf16,
                                       name=f"aogT{rp}_{qt}", tag="aogT")
                    aogT_map[(rp, qt)] = aogT
                    trp = pspr.tile([P, NCT, P], bf16,
                                    name=f"trp{rp}_{qt}", tag="pp")
                    # all 8 transposes first, THEN the psum->sbuf copies:
                    # a copy reads the whole trp tile (tile-granular deps),
                    # so interleaving would stall later transposes on it
                    for ct in range(NCT):
                        mm(trp[:, ct, :], lhsT=aog[:, ct, qt, :],
                           rhs=ident, is_transpose=True)
                        if ct % 2 == 1:
                            yield
                    nc.vector.tensor_copy(aogT[:, 0:NCT // 2, :],
                                          trp[:, 0:NCT // 2, :])
                    if use_act:
                        nc.scalar.activation(
                            out=aogT[:, NCT // 2:NCT, :],
                            in_=trp[:, NCT // 2:NCT, :],
                            func=mybir.ActivationFunctionType.Copy)
                    else:
                        nc.vector.tensor_copy(aogT[:, NCT // 2:NCT, :],
                                              trp[:, NCT // 2:NCT, :])
                    yield

            def gen_outproj(rp, chain):
                # one outproj chain (tok tile qt=chain//2, oc half=chain%2)
                # for round rp; gen_transposes(rp) must have been emitted
                qt, half = chain // 2, chain % 2
                if half == 0:
                    po_map[(rp, qt)] = fpool.tile(
                        [P, C], f32, name=f"po{rp}_{qt}", tag="po")
                aogT = aogT_map[(rp, qt)]
                po = po_map[(rp, qt)]
                pp = pspr.tile([P, TQ], f32, name=f"pop{rp}_{qt}_{half}",
                               tag="pp")
                for ct in range(NCT):
                    mm(pp,
                       lhsT=aogT[:, ct, :],
                       rhs=wp_sb[ct][:, half * TQ:(half + 1) * TQ],
                       start=(ct == 0), stop=(ct == NCT - 1))
                    yield
                rows = slice(rp * 2 * P + qt * P, rp * 2 * P + (qt + 1) * P)
                if rp == NQ - 1:
                    # tail-critical: split the psum drain across DVE+ACT
                    # and alternate hwdge queues across chains (the per-dma
                    # SP.SEQ issue cost dominates, so don't split the dma)
                    hq = TQ // 2
                    nc.vector.tensor_copy(
                        po[:, half * TQ:half * TQ + hq], pp[:, 0:hq])
                    nc.scalar.activation(
                        out=po[:, half * TQ + hq:(half + 1) * TQ],
                        in_=pp[:, hq:TQ],
                        func=mybir.ActivationFunctionType.Copy)
                    if qt == 1 and half == 1:
                        # very last store: two half-dmas on separate
                        # queues, each gated only on its own copy half
                        nc.sync.dma_start(
                            out=out[rows, half * TQ:half * TQ + hq],
                            in_=po[:, half * TQ:half * TQ + hq])
                        nc.scalar.dma_start(
                            out=out[rows, half * TQ + hq:(half + 1) * TQ],
                            in_=po[:, half * TQ + hq:(half + 1) * TQ])
                    else:
                        eng = nc.sync if half == 0 else nc.scalar
                        eng.dma_start(
                            out=out[rows, half * TQ:(half + 1) * TQ],
                            in_=po[:, half * TQ:(half + 1) * TQ])
                else:
                    nc.vector.tensor_copy(po[:, half * TQ:(half + 1) * TQ],
                                          pp)
                    nc.sync.dma_start(out=out[rows,
                                              half * TQ:(half + 1) * TQ],
                                      in_=po[:, half * TQ:(half + 1) * TQ])

            # x blocks load one (r, b) block ahead of the fillers that
            # consume them so the first filler matmul never waits on dma
            blocks = [(r, b) for r in range(NQ) for b in range(B)]
            xq_tiles = {}

            def stage_xq(i):
                if i >= len(blocks):
                    return
                r_, b_ = blocks[i]
                if r_ + 1 < NQ and (b_, r_ + 1) not in xq_tiles:
                    xq_tiles[(b_, r_ + 1)] = load_xq(b_, r_ + 1, nc.sync)

            stage_xq(0)
            cc_ins = []
            xq_v = {}
            pend = []
            post_stage = []
            for r in range(NQ):
                ntk = (r + 1) * QTPC
                for b in range(B):
                    stage_xq(B * r + b + 1)
                    # chain proj fillers (next chunk) and outproj fillers
                    # (round r-2, whose AllToAll landed during chunk r-1)
                    gens = []
                    if r == 0 and b < B - 1:
                        # next batch's chunk-0 projections lead so block
                        # (0, b+1) finds its qT/kT/v ready
                        gens.append(proj_all(b + 1, 0, xq0[b + 1]))
                    if r + 1 < NQ - 1:
                        gens.append(proj_all(b, r + 1,
                                             xq_tiles.pop((b, r + 1))))
                    elif r + 1 == NQ - 1:
                        # chunk 3's FULL projection (incl. v) runs here:
                        # the v yields feed chunk 2's starved diagonal
                        # tail, and chunk 3's blocks become chains-only
                        xq = xq_tiles.pop((b, r + 1))
                        gens.append(proj_all(b, r + 1, xq))
                        if b == 1:
                            # round-0 gather landed during chunk 1; flip it
                            # feature-major while chunk 2 runs
                            gens.append(gen_transposes(0))
                    elif r == NQ - 1:
                        # chunk 3 is exp-paced: outproj chains for rounds
                        # 0/1 are its fillers (v was projected in chunk 2)
                        if b == 1:
                            gens.append(gen_transposes(1))
                        chains_by_b = {0: [(0, 0), (0, 1)],
                                       1: [(0, 2), (0, 3)],
                                       2: [(1, 0), (1, 1)],
                                       3: [(1, 2), (1, 3)]}
                        for rp_, ch_ in chains_by_b[b]:
                            gens.append(gen_outproj(rp_, ch_))
                    if r == 2 and b == 0:
                        outproj_reloads(0, cc_ins[0])
                    elif r == 3 and b == 1:
                        # issue the reloads early (cc2 is only emitted at
                        # (3,0) tile 3 via the deferred staging); round 2
                        # outproj runs AFTER round 3 is posted
                        outproj_reloads(1, cc_ins[1])
                        outproj_reloads(2, cc_ins[2])
                    filler = (x for g in gens for x in g) if gens else None
                    if r == 0 and b == 2 and not wp_sb:
                        wp_sb.extend(load_w(w_p, "wp", C, nc.scalar))

                    # transposed attn@v accumulators: [query, feat] per two
                    # query blocks sharing one psum bank.  HW psum start
                    # semantics are epoch-like per bank: start=True bumps
                    # the bank epoch and stale-epoch cells OVERWRITE on
                    # their next write -- so each bank must see exactly ONE
                    # start (its very first matmul) per block, with every
                    # other matmul accumulating in that epoch.
                    av_t = [psav.tile([P, 2, 2 * P], f32,
                                      name=f"av{x_}_{r}_{b}",
                                      tag=f"av{x_}")
                            for x_ in range(2)]
                    # normalized token-major attn out, laid out
                    # [parity, pos, head, HD] so each a2a block is one
                    # contiguous [P, 2, 128] dma; per-query reciprocals
                    nao = npool.tile([P, 2, 2, 2, HD], bf16,
                                     name=f"nao_{r}_{b}", tag="nao")
                    rec = npool.tile([P, QTPC, 2, 1], f32,
                                     name=f"rec_{r}_{b}", tag="rec")

                    def drain_qt(qt, av_t=av_t, nao=nao, rec=rec):
                        # denominator is col HD of each head's 65-col group:
                        # per-query-partition scalar -> tensor_scalar
                        t_, sl = av_t[qt // 2], qt % 2
                        nq_ = nao[:, qt % 2, qt // 2]
                        seg = t_[:, sl, 0:2 * (HD + 1)].rearrange(
                            "p (h x) -> p h x", h=2)
                        nc.vector.reciprocal(rec[:, qt, :, 0],
                                             seg[:, :, HD])
                        nc.vector.tensor_scalar_mul(
                            nq_[:, 0, :], t_[:, sl, 0:HD],
                            rec[:, qt, 0, :])
                        nc.vector.tensor_scalar_mul(
                            nq_[:, 1, :], t_[:, sl, HD + 1:2 * HD + 1],
                            rec[:, qt, 1, :])

                    def emit_av(tk, e, c0, r=r, b=b, av_t=av_t,
                                drain_qt=drain_qt):
                        for qt in range(c0 // P, QTPC):
                            t_, sl = av_t[qt // 2], qt % 2
                            last = (tk == r * QTPC + qt)
                            for h in range(2):
                                # exactly one start (first mm into the
                                # bank) and one stop (last mm: the odd
                                # qt's head-B on its causal-stop tile)
                                mm(t_[:, sl,
                                      h * (HD + 1):(h + 1) * (HD + 1)],
                                   lhsT=e[:, h, qt * P:(qt + 1) * P],
                                   rhs=v_sb[b][tk][:, h, :],
                                   start=(tk == 0 and qt % 2 == 0
                                          and h == 0),
                                   stop=(last and qt % 2 == 1 and h == 1),
                                   skip_group_check=True)
                            # drains read the whole psum tile (tile-granular
                            # deps), so only drain once BOTH query blocks of
                            # a tile have stopped -- no write-after-read
                            # stall on the still-accumulating block
                            if last and qt in (1, 3):
                                drain_qt(qt - 1)
                                drain_qt(qt)

                    def fill(n, pad=False):
                        for _ in range(n):
                            if filler is None or \
                                    next(filler, "done") == "done":
                                if pad and pad_budget[0] > 0:
                                    # emergency: break the PE<->ACT
                                    # serialization with a junk matmul
                                    pad_budget[0] -= 1
                                    dmy = pspr.tile(
                                        [P, TQ], f32,
                                        name=f"dmf_{r}_{b}_{pad_budget[0]}",
                                        tag="pp")
                                    mm(dmy[:, 0:P], lhsT=junk[:, 0:P],
                                       rhs=junk[:, 0:P], start=True,
                                       stop=True)
                                else:
                                    break

                    # stage this batch's two slices (parity 0/1) token-major
                    # straight from nao: block j=2b+p rows [256j, 256j+256)
                    # of a2a_in[r] = [2 qtiles x 128 tokens, my 128 feats].
                    # Deferred into the NEXT block's tile-3 slot: the attn@v
                    # pend queue is carried across blocks, so this block's
                    # last nao writes are only emitted there.
                    def do_stage(r=r, b=b, nao=nao, rec=rec):
                        if nao_dump is not None:
                            nc.gpsimd.dma_start(out=nao_dump[r, b],
                                                in_=nao)
                            nc.gpsimd.dma_start(out=rec_dump[r, b],
                                                in_=rec)
                        a2r = a2a_in[r][2 * b * 2 * P:(2 * b + 2) * 2 * P, :]
                        a2r = a2r.rearrange("(x tt p) c -> x p tt c",
                                            x=2, p=P)
                        if r == NQ - 1 and b == B - 1:
                            # last block: stage on the two parallel hwdge
                            # queues (nothing follows them there) so the
                            # final AllToAll posts sooner
                            nc.sync.dma_start(
                                out=a2r[0],
                                in_=nao[:, 0].rearrange(
                                    "p t h d -> p t (h d)"))
                            nc.scalar.dma_start(
                                out=a2r[1],
                                in_=nao[:, 1].rearrange(
                                    "p t h d -> p t (h d)"))
                        else:
                            for xb in range(2):
                                nc.gpsimd.dma_start(
                                    out=a2r[xb],
                                    in_=nao[:, xb].rearrange(
                                        "p t h d -> p t (h d)"))
                        if b == B - 1:
                            cc = nc.gpsimd.collective_compute(
                                "AllToAll",
                                BYP,
                                replica_groups=GROUP8,
                                ins=[a2a_in[r][:].opt()],
                                outs=[a2a_out[r][:].opt()],
                            )
                            cc_ins.append(cc.ins)

                    # attn@v for tile tk is emitted after the scores for
                    # tk+3 so the in-order PE queue never waits on exp;
                    # projection-chain fillers absorb the exp-paced bubbles
                    pad_budget = [12]
                    for tk in range(ntk):
                        ks = slice(tk * P, (tk + 1) * P)
                        m = max(0, tk - r * QTPC)
                        c0 = P * m
                        qsm = slice(c0, TQ)
                        diag = tk >= r * QTPC
                        s = psmm.tile([P, 2 * TQ], f32,
                                      name=f"s_{r}_{b}_{tk}", tag="sc")
                        mm(s[:, c0:TQ], lhsT=kT[b][0:HD, ks],
                           rhs=qT[b][r][0:HD, qsm], start=True, stop=True)
                        mm(s[:, TQ + c0:2 * TQ], lhsT=kT[b][HD:P, ks],
                           rhs=qT[b][r][HD:P, qsm], start=True, stop=True)
                        e = epool.tile([P, 2, TQ], bf16,
                                       name=f"e_{r}_{b}_{tk}", tag="e")
                        # exp only over the computed columns [c0:TQ] of both
                        # heads (strided); the skipped region is never read
                        nc.scalar.activation(
                            out=e[:, :, c0:TQ],
                            in_=s.rearrange("p (h q) -> p h q", h=2)[:, :,
                                                                     c0:TQ],
                            func=EXP, scale=SCALE)
                        if diag:
                            # triangle-mask only the diagonal 128-col block
                            nc.vector.tensor_mul(e[:, :, c0:c0 + P],
                                                 e[:, :, c0:c0 + P],
                                                 mask_tri)
                        pend.append((emit_av, tk, e, c0))
                        # force-drain the previous block's carried entries
                        # in the first two tiles (before this tile's
                        # fillers!) so its psum drains (DVE) land ahead of
                        # the filler-chain copies in the DVE queue and well
                        # before this block's first av write
                        want = 2 if tk < 2 else 4
                        while len(pend) > want:
                            f_, *a_ = pend.pop(0)
                            f_(*a_)
                        fill(4 if diag else 2, pad=diag)
                        if tk == 3 and post_stage:
                            # previous block's nao is fully written by now
                            post_stage.pop(0)()
                    if r == NQ - 1 and b == B - 1:
                        # flush with dep-free junk pads only: real fillers
                        # would emit DVE copies ahead of the final drains
                        # and delay the last AllToAll post
                        for fi_ in range(len(pend)):
                            dmy = pspr.tile([P, TQ], f32,
                                            name=f"dmg{fi_}", tag="pp")
                            mm(dmy[:, 0:P], lhsT=junk[:, 0:P],
                               rhs=junk[:, 0:P], start=True, stop=True)
                            f_, *a_ = pend.pop(0)
                            f_(*a_)
                        # post the final AllToAll FIRST; leftover outproj
                        # fillers then run during its latency
                        do_stage()
                        fill(1000)
                    else:
                        # drain remaining fillers for the next chunk
                        fill(1000)
                        post_stage.append(do_stage)
            # round 2's second half runs during round 3's collective,
            # followed by dummy matmuls that keep the PE p-state warm until
            # the gathered data lands
            outproj_reloads(NQ - 1, cc_ins[NQ - 1], split=True)
            for _ in gen_transposes(2, use_act=True):
                pass
            for chain in range(4):
                for _ in gen_outproj(2, chain):
                    pass
            for i in range(110):
                dmy = pspr.tile([P, TQ], f32, name=f"dmy{i}", tag="pp")
                mm(dmy, lhsT=wq_sb[0], rhs=kT[0][:, 0:TQ],
                   start=True, stop=True)
            for i in range(20):
                dmy = pspr.tile([P, TQ], f32, name=f"dmz{i}", tag="pp")
                mm(dmy[:, 0:P], lhsT=junk[:, 0:P], rhs=junk[:, 0:P],
                   start=True, stop=True)
            # final output projection (round NQ-1)
            for _ in gen_transposes(NQ - 1, use_act=True):
                pass
            for chain in range(4):
                for _ in gen_outproj(NQ - 1, chain):
                    pass

    if not nc.is_finalized():
        nc.finalize()
    return nc


def _get_nc():
    if "nc" not in _NC_CACHE:
        _NC_CACHE["nc"] = _build_nc()
    return _NC_CACHE["nc"]


def kernel(x, w_qkv, w_proj):
    import ml_dtypes
    from concourse.bass_utils import run_bass_kernel_spmd

    bf = ml_dtypes.bfloat16
    x = np.asarray(x, dtype=np.float32)
    w_qkv = np.asarray(w_qkv, dtype=np.float32)
    w_proj = np.asarray(w_proj, dtype=np.float32)

    xT = np.ascontiguousarray(x.transpose(2, 0, 1)).astype(bf)  # [C, B, S]
    wp = np.ascontiguousarray(w_proj).astype(bf)
    in_maps = []
    for j in range(N_CORES):
        fs = slice(F_LOC * j, F_LOC * (j + 1))
        in_maps.append({
            "x_t": xT,
            "w_q": np.ascontiguousarray(w_qkv[:, 0 * C:1 * C][:, fs]).astype(bf),
            "w_k": np.ascontiguousarray(w_qkv[:, 1 * C:2 * C][:, fs]).astype(bf),
            "w_v": np.ascontiguousarray(w_qkv[:, 2 * C:3 * C][:, fs]).astype(bf),
            "w_p": wp,
        })

    res = run_bass_kernel_spmd(_get_nc(), in_maps,
                               core_ids=list(range(N_CORES)))
    _NC_CACHE["last_res"] = res

    # core j computed tokens {qtile j%2 + 2i} of batch j//2, all channels
    out = np.empty((B, S, C), dtype=np.float32)
    for j in range(N_CORES):
        b, p_ = j // 2, j % 2
        o = res.results[j]["out"]  # [1024, 1024]
        for i in range(S // (2 * P)):
            g = p_ + 2 * i
            out[b, g * P:(g + 1) * P, :] = o[i * P:(i + 1) * P, :]
    return out

